# revision 89
# baseline (speedup 1.0000x reference)
"""Trainium2 Bass kernel for nn_AttentionBlock (GroupNorm + qkv conv + head-dim attention + proj + residual).

Sharding: data-parallel over batch B=16 -> 2 batch elements per core on 8 cores.

Structure (per batch element):
  The attention here contracts over PIXELS (scores are [64,64] per head), so
  q,k are never materialized per-pixel. Instead:
    G   = X X^T            [512,512] Gram matrix of raw x (PE transposes + f32r matmuls)
    Tk  = G Wk'^T + Sx (X) Bk^T      (rank-1 bias fold)
    S_p = Wq'^T Tk + Bq (X) hk_full^T (rank-1; off-diagonal head blocks unused)
  GroupNorm is folded into the weights (Wq' = Wq diag(a), biases via b2), so x
  itself is never normalized. v/proj/attn-v follow the dense path; residual is
  re-read from the x tiles already in SBUF (no second DMA).
"""
import sys, os
sys.path.insert(0, "/opt/trn_rl_repo")
sys.path.insert(0, "/opt/trn_rl_repo/concourse")
import numpy as np

B, C, H, W = 16, 512, 64, 64
N = H * W            # 4096 spatial
NH = 8               # heads
D = C // NH          # 64 head dim
G = 32               # groups
EPS = 1e-5
NCORES = 8
BPC = B // NCORES    # 2 batches per core

NT = C // 128        # 4 channel tiles
NCHUNK = N // 128    # 32 pixel chunks
NJ = N // 512        # 8 column blocks of 512

_cache = {}


def _build():
    import concourse.bass as bass
    import concourse.bacc as bacc
    import concourse.tile as tile
    from concourse import mybir
    from concourse.masks import make_identity

    f32 = mybir.dt.float32
    f32r = mybir.dt.float32r
    bf16 = mybir.dt.bfloat16
    AF = mybir.ActivationFunctionType
    ALU = mybir.AluOpType
    AX = mybir.AxisListType

    nc = bacc.Bacc()

    x2 = nc.dram_tensor("x2", [BPC, C, N], f32r, kind="ExternalInput")
    wqkT = nc.dram_tensor("wqkT", [C, 3 * C], f32r, kind="ExternalInput")  # w_qkv.T  [c, o]
    wpT = nc.dram_tensor("wpT", [C, C], f32, kind="ExternalInput")          # w_proj.T [c, o]
    gamma_pc = nc.dram_tensor("gamma_pc", [128, NT], f32, kind="ExternalInput")
    beta_pc = nc.dram_tensor("beta_pc", [128, NT], f32, kind="ExternalInput")
    bqk_row = nc.dram_tensor("bqk_row", [1, 2 * C], f32, kind="ExternalInput")
    bv_pc = nc.dram_tensor("bv_pc", [128, NT], f32, kind="ExternalInput")
    bp_pc = nc.dram_tensor("bp_pc", [128, NT], f32, kind="ExternalInput")
    ident_d = nc.dram_tensor("ident_d", [128, 128], f32r, kind="ExternalInput")
    gmask_d = nc.dram_tensor("gmask_d", [128, 8], f32r, kind="ExternalInput")
    gmaskT_d = nc.dram_tensor("gmaskT_d", [8, 128], f32r, kind="ExternalInput")
    out2 = nc.dram_tensor("out2", [BPC, C, N], f32, kind="ExternalOutput")

    with tile.TileContext(nc) as tc:
        with tc.tile_pool(name="consts", bufs=1) as consts, \
             tc.tile_pool(name="wpool", bufs=1) as wpool, \
             tc.tile_pool(name="xpool", bufs=1) as xpool, \
             tc.tile_pool(name="gpool", bufs=1) as gpool, \
             tc.tile_pool(name="xtcpool", bufs=6) as xtcpool, \
             tc.tile_pool(name="rows", bufs=1) as rows, \
             tc.tile_pool(name="work", bufs=2) as work, \
             tc.tile_pool(name="stagepool", bufs=2) as stagepool, \
             tc.tile_pool(name="ps", bufs=1, space="PSUM") as ps:

            # ---------------- constants / weights (once per core) ----------------
            ident = consts.tile([128, 128], f32, tag="ident")
            make_identity(nc, ident)
            identr = consts.tile([128, 128], f32r, tag="identr")
            nc.scalar.dma_start(out=identr, in_=ident_d[:, :])
            identb = consts.tile([128, 128], bf16, tag="identb")
            nc.gpsimd.dma_start(out=identb, in_=ident_d[:, :])
            gmask = consts.tile([128, 8], f32r, tag="gmask")
            nc.scalar.dma_start(out=gmask, in_=gmask_d[:, :])
            gmaskT = consts.tile([8, 128], f32r, tag="gmaskT")
            nc.scalar.dma_start(out=gmaskT, in_=gmaskT_d[:, :])

            bpc_t = consts.tile([128, NT], f32, tag="bpc_t")
            nc.scalar.dma_start(out=bpc_t, in_=bp_pc[:, :])
            gam = consts.tile([128, NT], f32, tag="gam")
            bet = consts.tile([128, NT], f32, tag="bet")
            bvc = consts.tile([128, NT], f32, tag="bvc")
            bqkr = consts.tile([1, 2 * C], f32, tag="bqkr")
            nc.sync.dma_start(out=gam, in_=gamma_pc[:, :])
            nc.sync.dma_start(out=bet, in_=beta_pc[:, :])
            nc.sync.dma_start(out=bvc, in_=bv_pc[:, :])
            nc.sync.dma_start(out=bqkr, in_=bqk_row[:, :])

            ws = []
            wp = []
            for t in range(NT):
                ws_t = wpool.tile([128, 3 * C], f32r, tag=f"ws{t}", name=f"ws{t}")
                ws.append(ws_t)
                w_p = wpool.tile([128, C], bf16, tag=f"wp{t}", name=f"wp{t}")
                wp.append(w_p)

            for b in range(BPC):
                # ---------------- load x (fp32 -> f32r cast DMA) ----------------
                # Quarter-granular tiles: WAR deps resolve per quarter, so the next
                # batch's loads start as soon as this batch's tail frees a quarter.
                # b=0 may use sync (out2 writes haven't started); b=1 must avoid the
                # sync queue so its x loads aren't stuck behind b=0's out2 writes.
                # Emission q-major so quarters land in the order Gx consumes them.
                x_engines = [nc.sync, nc.gpsimd] if b == 0 else [nc.gpsimd]
                wtmp_eng = nc.sync if b == 0 else nc.gpsimd
                xq = [[None] * 4 for _ in range(NT)]
                wtmp_l = []
                for q in range(4):
                    for t in range(NT):
                        x_qt = xpool.tile([128, 1024], f32r, tag=f"x{t}q{q}", name=f"x{t}q{q}")
                        if q > 0:
                            x_engines[(q * 4 + t) % len(x_engines)].dma_start(
                                out=x_qt,
                                in_=x2[b, 128 * t:128 * (t + 1), 1024 * q:1024 * (q + 1)])
                        xq[t][q] = x_qt
                    if q == 0:
                        # first quarter lands chunk-by-chunk so transposes start early
                        for c in range(8):
                            for t in range(NT):
                                x_engines[(c * 4 + t) % len(x_engines)].dma_start(
                                    out=xq[t][0][:, 128 * c:128 * (c + 1)],
                                    in_=x2[b, 128 * t:128 * (t + 1), 128 * c:128 * (c + 1)])
                for t in range(NT):
                    wtmp = work.tile([128, 3 * C], f32r, tag="wtmp", name="wtmp", bufs=4)
                    wtmp_eng.dma_start(out=wtmp, in_=wqkT[128 * t:128 * (t + 1), :])
                    wtmp_l.append(wtmp)
                if b == 0:
                    for t in range(NT):
                        nc.gpsimd.dma_start(out=wp[t], in_=wpT[128 * t:128 * (t + 1), :])

                e_sl = [work.tile([128, 128], bf16, tag=f"es{p}", name=f"es{p}", bufs=1)
                        for p in range(4)]

                # triangular Gram blocks: row-block cb covers cols 128*cb..511
                gx_tags = ["half", "half", "aps", "aps"]
                gx_w = [512, 384, 256, 256]
                gx = [ps.tile([128, gx_w[cb]], f32, tag=gx_tags[cb], name=f"gx{cb}", bufs=2)
                      for cb in range(4)]

                # ---- stats / bias blocks, emitted interleaved into the chunk loop
                # so the PE stream never head-of-line blocks on their inputs ----
                acol = work.tile([128, NT], f32, tag="acol")
                epst8 = work.tile([8, 1], f32, tag="epst8")
                nc.vector.memset(epst8, EPS)
                bsx_l = []
                sxrow_l = []
                mv_l = []

                def emit_bn(t):
                    # DVE-only part; emitted early (per-quarter deps)
                    st = work.tile([128, 8, 6], f32, tag="bnstats")
                    for j in range(8):
                        xfq = xq[t][j // 2].bitcast(f32)
                        nc.vector.bn_stats(out=st[:, j, :], in_=xfq[:, 512 * (j % 2):512 * (j % 2 + 1)])
                    mv = work.tile([128, 2], f32, tag="mv", bufs=4)
                    nc.vector.bn_aggr(out=mv, in_=st)
                    mv_l.append(mv)

                def emit_stats(t):
                    # Group aggregation via mask matmuls: gsum = gmask^T @ [mean, var,
                    # mean^2]; per-channel broadcast back via gmaskT^T @ [mean_g, rstd_g].
                    mv = mv_l[t]
                    rhs3 = work.tile([128, 4], f32r, tag="rhs3")
                    nc.vector.tensor_copy(rhs3[:, 0:2], mv)
                    nc.scalar.activation(out=rhs3[:, 2:4], in_=mv, func=AF.Square)
                    gsum_ps = ps.tile([8, 4], f32, tag="stats", name="gsum_ps", bufs=1)
                    nc.tensor.matmul(gsum_ps, gmask, rhs3, start=True, stop=True)
                    gsum = work.tile([8, 4], f32, tag="gsum")
                    nc.vector.tensor_copy(gsum, gsum_ps)
                    mean_g = work.tile([8, 1], f32, tag="mean_g")
                    nc.scalar.mul(out=mean_g, in_=gsum[:, 0:1], mul=1.0 / 16.0)
                    ex2 = work.tile([8, 1], f32, tag="ex2")
                    nc.vector.tensor_add(ex2, gsum[:, 1:2], gsum[:, 2:3])
                    msq = work.tile([8, 1], f32, tag="msq")
                    nc.scalar.activation(out=msq, in_=mean_g, func=AF.Square)
                    var_g = work.tile([8, 1], f32, tag="var_g")
                    nc.vector.tensor_scalar(out=var_g, in0=ex2, scalar1=1.0 / 16.0, scalar2=None, op0=ALU.mult)
                    nc.vector.tensor_tensor(var_g, var_g, msq, op=ALU.subtract)
                    sdg = work.tile([8, 1], f32, tag="sdg")
                    nc.scalar.activation(out=sdg, in_=var_g, func=AF.Sqrt, bias=epst8)
                    rhs2 = work.tile([8, 4], f32r, tag="rhs2")
                    with nc.allow_low_precision(reason="feeds f32r matmul which rounds anyway"):
                        nc.vector.reciprocal(rhs2[:, 1:2], sdg)
                        nc.vector.reciprocal(rhs2[:, 3:4], sdg)
                    nc.vector.tensor_copy(rhs2[:, 0:1], mean_g)
                    nc.vector.tensor_copy(rhs2[:, 2:3], mean_g)
                    pcmr = ps.tile([128, 4], f32, tag="stats", name="pcmr", bufs=1)
                    nc.tensor.matmul(pcmr, gmaskT, rhs2, start=True, stop=True)
                    nc.vector.tensor_tensor(acol[:, t:t + 1], pcmr[:, 1:2], gam[:, t:t + 1], op=ALU.mult)
                    sxtp = ps.tile([1, 128], f32, tag="stats", name="sxtp", bufs=1)
                    nc.tensor.transpose(sxtp, mv[:, 0:1], ident)
                    sxrow = rows.tile([1, 128], f32r, tag=f"sxrow{t}", name=f"sxrow{t}")
                    nc.scalar.mul(out=sxrow, in_=sxtp, mul=float(N))
                    sxrow_l.append(sxrow)
                    # bsx: col0 = b2 = beta - mean*a ; col1 = b2 + a*raw_channel_mean
                    bsx = rows.tile([128, 2], f32r, tag=f"bsx{t}", name=f"bsx{t}")
                    tmpb = work.tile([128, 1], f32, tag="tmpb")
                    nc.vector.tensor_tensor(tmpb, pcmr[:, 0:1], acol[:, t:t + 1], op=ALU.mult)
                    nc.vector.tensor_tensor(bsx[:, 0:1], bet[:, t:t + 1], tmpb, op=ALU.subtract)
                    amv = work.tile([128, 1], f32, tag="amv")
                    nc.vector.tensor_tensor(amv, acol[:, t:t + 1], mv[:, 0:1], op=ALU.mult)
                    nc.vector.tensor_tensor(bsx[:, 1:2], bsx[:, 0:1].bitcast(f32), amv, op=ALU.add)
                    bsx_l.append(bsx)
                    nc.vector.tensor_scalar_mul(out=ws[t], in0=wtmp_l[t].bitcast(f32), scalar1=acol[:, t:t + 1])

                # bias rows (scoresA bank rotation, one at a time)
                vbias = work.tile([128, NT], f32r, tag="vbias")
                brow_out = {}

                def emit_biasrow(r):
                    lcol = [0, 0, 0, 1][r]
                    rlo = [2 * C, 0, 512, 512][r]
                    row_ps = ps.tile([1, 512], f32, tag="scoresA", name=f"brow{r}_ps", bufs=1)
                    for t in range(NT):
                        nc.tensor.matmul(row_ps, bsx_l[t][:, lcol:lcol + 1],
                                         wtmp_l[t][:, rlo:rlo + 512],
                                         start=(t == 0), stop=(t == NT - 1), skip_group_check=True)
                    if r == 0:
                        vbrow = rows.tile([1, 512], f32, tag="vbrow")
                        nc.scalar.copy(vbrow, row_ps)
                        for m in range(NT):
                            vtp = ps.tile([128, 1], f32, tag="stats", name="vtp", bufs=1)
                            nc.tensor.transpose(vtp, vbrow[:, 128 * m:128 * (m + 1)], ident[0:1, 0:1])
                            nc.vector.tensor_add(vbias[:, m:m + 1], vtp, bvc[:, m:m + 1])
                    elif r == 1:
                        browq = rows.tile([1, 512], f32r, tag="browq")
                        nc.vector.tensor_add(browq, row_ps, bqkr[:, 0:512])
                        brow_out["q"] = browq
                    elif r == 2:
                        browk = rows.tile([1, 512], f32r, tag="browk")
                        nc.vector.tensor_add(browk, row_ps, bqkr[:, 512:1024])
                        brow_out["k"] = browk
                    else:
                        hkpre = rows.tile([1, 512], f32, tag="hkpre")
                        nc.vector.tensor_add(hkpre, row_ps, bqkr[:, 512:1024])
                        hkf = rows.tile([1, 512], f32r, tag="hkf")
                        nc.scalar.mul(out=hkf, in_=hkpre, mul=float(N))
                        brow_out["hkf"] = hkf

                # ---------------- transposes + Gram matrix (upper triangle) ----------------
                def emit_gx(xtc_prev, nis):
                    for cb in range(4):
                        nc.tensor.matmul(gx[cb], xtc_prev[:, 128 * cb:128 * (cb + 1)],
                                         xtc_prev[:, 512 - gx_w[cb]:512],
                                         start=(nis == 0), stop=(nis == NCHUNK - 1),
                                         skip_group_check=True)

                for t in range(NT):
                    emit_bn(t)
                prev_xtc = None
                for ni in range(NCHUNK):
                    tps = ps.tile([128, 512], f32r, tag="trans", name="tps", bufs=2)
                    for t in range(NT):
                        nc.tensor.transpose(tps[:, 128 * t:128 * (t + 1)],
                                            xq[t][ni // 8][:, 128 * (ni % 8):128 * (ni % 8 + 1)],
                                            identr)
                    xtc = xtcpool.tile([128, 512], f32r, tag="xtc")
                    nc.scalar.copy(xtc, tps.bitcast(f32))
                    if prev_xtc is not None:
                        emit_gx(prev_xtc, ni - 1)
                    prev_xtc = xtc
                    if ni == 22:
                        for t in range(NT):
                            emit_stats(t)
                    elif ni in (25, 27, 29, 31):
                        emit_biasrow((ni - 25) // 2)
                emit_gx(prev_xtc, NCHUNK - 1)
                browq, browk, hkf = brow_out["q"], brow_out["k"], brow_out["hkf"]

                # G -> SBUF (split across ACT and DVE to halve the drain latency)
                gs = []
                for cb in range(4):
                    g_s = gpool.tile([128, gx_w[cb]], f32r, tag=f"gs{cb}", name=f"gs{cb}")
                    if cb % 2 == 0:
                        nc.scalar.activation(out=g_s, in_=gx[cb], func=AF.Identity)
                    else:
                        nc.vector.tensor_copy(g_s, gx[cb])
                    gs.append(g_s)

                # lower-triangle blocks for Tk come from transposed upper blocks
                gT = {}
                for i, (cpb, cb) in enumerate([(1, 0), (2, 0), (3, 0), (2, 1), (3, 1), (3, 2)]):
                    blk = gs[cb][:, 128 * (cpb - cb) + gx_w[cb] - (512 - 128 * cb):128 * (cpb - cb) + gx_w[cb] - (512 - 128 * cb) + 128]
                    gtp = ps.tile([128, 128], f32r, tag="trans", name="gtp", bufs=2)
                    nc.tensor.transpose(gtp, blk, identr)
                    g_t = gpool.tile([128, 128], f32r, tag=f"gt{cpb}{cb}", name=f"gt{cpb}{cb}")
                    if i % 2 == 0:
                        nc.scalar.copy(g_t, gtp)
                    else:
                        nc.vector.tensor_copy(g_t, gtp)
                    gT[(cpb, cb)] = g_t

                def g_stat(cpb, cb):
                    if cpb <= cb:
                        off = 128 * cb - (512 - gx_w[cpb])
                        return gs[cpb][:, off:off + 128]
                    return gT[(cpb, cb)]

                # ---------------- Tk = G Wk'^T + Sx (x) Bk ----------------
                tks = []
                for cb in range(4):
                    tk = ps.tile([128, 512], f32, tag="aps", name=f"tk{cb}", bufs=2)
                    for cpb in range(4):
                        nc.tensor.matmul(tk, g_stat(cpb, cb),
                                         ws[cpb][:, 512:1024], start=(cpb == 0), stop=False)
                    nc.tensor.matmul(tk, sxrow_l[cb], browk, start=False, stop=True)
                    t_s = gpool.tile([128, 512], f32r, tag=f"tks{cb}", name=f"tks{cb}")
                    nc.scalar.activation(out=t_s, in_=tk, func=AF.Identity)
                    tks.append(t_s)

                # ---------------- scores (head pairs, diag blocks used) ----------------
                # Per-pair PSUM tiles in momentarily-free banks so each pair's
                # softmax starts as soon as its own accumulation stops.
                sc_tags = [("scoresA", 1), ("stats", 1), ("trans", 2), ("trans", 2)]
                scps_l = []
                for p in range(4):
                    stag, sbufs = sc_tags[p]
                    scp = ps.tile([128, 128], f32, tag=stag, name=f"scps{p}", bufs=sbufs)
                    for cb in range(4):
                        nc.tensor.matmul(scp, ws[cb][:, 128 * p:128 * (p + 1)],
                                         tks[cb][:, 128 * p:128 * (p + 1)],
                                         start=(cb == 0), stop=False, skip_group_check=True)
                    nc.tensor.matmul(scp, browq[:, 128 * p:128 * (p + 1)],
                                     hkf[:, 128 * p:128 * (p + 1)], start=False, stop=True,
                                     skip_group_check=True)
                    scps_l.append(scp)

                # ---------------- wsvT: transpose of the v-weight blocks ----------------
                # wsvT[p][d, c] = Wv'[d, c] for d in head-pair p (d-partitioned)
                wsvT = []
                for p in range(4):
                    wtps = ps.tile([128, 512], f32r, tag="trans", name="wtps", bufs=2)
                    for t in range(NT):
                        nc.tensor.transpose(wtps[:, 128 * t:128 * (t + 1)],
                                            ws[t][:, 2 * C + 128 * p:2 * C + 128 * (p + 1)], identr)
                    wsv_p = gpool.tile([128, 512], f32r, tag=f"wsvT{p}", name=f"wsvT{p}")
                    nc.scalar.copy(wsv_p, wtps.bitcast(f32))
                    wsvT.append(wsv_p)

                # ---------------- softmax (per head pair) -> rden-scaled E ----------------
                rden = work.tile([128, 4], f32, tag="rden")
                for p in range(4):
                    mx = work.tile([128, 1], f32, tag="mx")
                    nc.vector.reduce_max(out=mx[0:64, :], in_=scps_l[p][0:64, 0:64], axis=AX.X)
                    nc.vector.reduce_max(out=mx[64:128, :], in_=scps_l[p][64:128, 64:128], axis=AX.X)
                    negmx = work.tile([128, 1], f32, tag="negmx")
                    nc.scalar.mul(out=negmx, in_=mx, mul=-0.125)
                    e = work.tile([128, 128], f32, tag="exp")
                    nc.vector.memset(e, 0.0)
                    nc.scalar.activation(out=e[0:64, 0:64], in_=scps_l[p][0:64, 0:64],
                                         func=AF.Exp, scale=0.125, bias=negmx[0:64, :])
                    nc.scalar.activation(out=e[64:128, 64:128], in_=scps_l[p][64:128, 64:128],
                                         func=AF.Exp, scale=0.125, bias=negmx[64:128, :])
                    den = work.tile([128, 1], f32, tag="den")
                    nc.vector.reduce_sum(out=den[0:64, :], in_=e[0:64, 0:64], axis=AX.X)
                    nc.vector.reduce_sum(out=den[64:128, :], in_=e[64:128, 64:128], axis=AX.X)
                    nc.vector.reciprocal(rden[:, p:p + 1], den)
                    nc.scalar.activation(out=e_sl[p], in_=e, func=AF.Copy, scale=rden[:, p:p + 1])

                # ---------------- UT[d,o] = sum_c es[c,d] Wp[o,c] (per pair) ----------------
                uts = []
                for p in range(4):
                    ut_ps = ps.tile([128, 512], f32, tag="aps", name="ut_ps", bufs=2)
                    nc.tensor.matmul(ut_ps, e_sl[p], wp[p], start=True, stop=True)
                    ut_s = gpool.tile([128, 512], f32r, tag=f"uts{p}", name=f"uts{p}")
                    nc.scalar.activation(out=ut_s, in_=ut_ps, func=AF.Identity)
                    uts.append(ut_s)

                # ---------------- MT[c,o] = sum_d Wv'[d,c] UT[d,o] ----------------
                mts = []
                m_tags = ["half", "half", "aps", "aps"]
                for cb in range(4):
                    mt_ps = ps.tile([128, 512], f32, tag=m_tags[cb], name=f"mt_ps{cb}", bufs=2)
                    for p in range(4):
                        nc.tensor.matmul(mt_ps, wsvT[p][:, 128 * cb:128 * (cb + 1)], uts[p],
                                         start=(p == 0), stop=(p == 3))
                    mt_s = gpool.tile([128, 512], f32r, tag=f"mts{cb}", name=f"mts{cb}")
                    nc.scalar.activation(out=mt_s, in_=mt_ps, func=AF.Identity)
                    mts.append(mt_s)

                # ---------------- output bias col: bp + UT^T vb ----------------
                ob_ps = ps.tile([1, 512], f32, tag="stats", name="ob_ps", bufs=1)
                for p in range(4):
                    nc.tensor.matmul(ob_ps, vbias[:, p:p + 1], uts[p],
                                     start=(p == 0), stop=(p == 3), skip_group_check=True)
                obrow = rows.tile([1, 512], f32, tag="obrow")
                nc.scalar.copy(obrow, ob_ps)
                tbias = work.tile([128, NT], f32, tag="tbias")
                for m in range(NT):
                    obt = ps.tile([128, 1], f32, tag="stats", name="obt", bufs=1)
                    nc.tensor.transpose(obt, obrow[:, 128 * m:128 * (m + 1)], ident[0:1, 0:1])
                    nc.vector.tensor_add(tbias[:, m:m + 1], obt, bpc_t[:, m:m + 1])

                # ---------------- fused (v @ attn @ proj) GEMM + bias + residual ----------------
                pps_tags = [("half", 2), ("half", 2), ("scoresA", 1), ("stats", 1)]
                out_eng = [nc.sync, nc.sync]
                for nj in range(NJ):
                    qj, oj = nj // 2, 512 * (nj % 2)
                    for m in range(NT):
                        ptag, pbufs = pps_tags[m]
                        pps = ps.tile([128, 512], f32, tag=ptag, name="pps", bufs=pbufs)
                        for cb in range(4):
                            nc.tensor.matmul(pps, mts[cb][:, 128 * m:128 * (m + 1)],
                                             xq[cb][qj][:, oj:oj + 512],
                                             start=(cb == 0), stop=(cb == 3))
                        stage = stagepool.tile([128, 512], f32, tag="stage", bufs=4)
                        nc.vector.scalar_tensor_tensor(
                            out=stage, in0=pps, scalar=tbias[:, m:m + 1],
                            in1=xq[m][qj].bitcast(f32)[:, oj:oj + 512],
                            op0=ALU.add, op1=ALU.add)
                        out_eng[m % 2].dma_start(
                            out=out2[b, 128 * m:128 * (m + 1), 512 * nj:512 * (nj + 1)], in_=stage)

    nc.compile()
    return nc


def _get_nc():
    if "nc" not in _cache:
        _cache["nc"] = _build()
    return _cache["nc"]


def kernel(x, gamma, beta, w_qkv, b_qkv, w_proj, b_proj):
    from concourse.bass_utils import run_bass_kernel_spmd

    x = np.asarray(x, dtype=np.float32)
    gamma = np.asarray(gamma, dtype=np.float32)
    beta = np.asarray(beta, dtype=np.float32)
    w_qkv = np.asarray(w_qkv, dtype=np.float32)
    b_qkv = np.asarray(b_qkv, dtype=np.float32)
    w_proj = np.asarray(w_proj, dtype=np.float32)
    b_proj = np.asarray(b_proj, dtype=np.float32)

    nc = _get_nc()

    wqkT = np.ascontiguousarray(w_qkv.T)                       # [512, 1536]
    wpT = np.ascontiguousarray(w_proj.T)                       # [512, 512]
    gamma_pc = np.ascontiguousarray(gamma.reshape(NT, 128).T)  # [128, 4]
    beta_pc = np.ascontiguousarray(beta.reshape(NT, 128).T)
    bqk_row = np.ascontiguousarray(b_qkv[:2 * C].reshape(1, 2 * C))
    bv_pc = np.ascontiguousarray(b_qkv[2 * C:].reshape(NT, 128).T)
    bp_pc = np.ascontiguousarray(b_proj.reshape(NT, 128).T)
    ident_d = np.eye(128, dtype=np.float32)
    gmask_d = np.zeros((128, 8), dtype=np.float32)
    gmask_d[np.arange(128), np.arange(128) // 16] = 1.0
    gmaskT_d = np.ascontiguousarray(gmask_d.T)

    xr = x.reshape(B, C, N)
    in_maps = []
    for i in range(NCORES):
        in_maps.append({
            "x2": np.ascontiguousarray(xr[BPC * i:BPC * (i + 1)]),
            "wqkT": wqkT, "wpT": wpT,
            "gamma_pc": gamma_pc, "beta_pc": beta_pc,
            "bqk_row": bqk_row, "bv_pc": bv_pc, "bp_pc": bp_pc,
            "ident_d": ident_d, "gmask_d": gmask_d, "gmaskT_d": gmaskT_d,
        })

    res = run_bass_kernel_spmd(nc, in_maps, core_ids=list(range(NCORES)))
    out = np.empty((B, C, N), dtype=np.float32)
    for i in range(NCORES):
        out[BPC * i:BPC * (i + 1)] = res.results[i]["out2"]
    return out.reshape(B, C, H, W)
